# revision 5
# baseline (speedup 1.0000x reference)
"""Fused multi-head attention (QKV projection + softmax attention) on 8 TRN2
NeuronCores — v17.

Problem: x [2, 2048, 1024] f32, w_qkv [1024, 3072] f32 ->
         out [2, 16, 2048, 64] f32   (16 heads, head_dim 64)

Sharding: batch x head-group. Core c handles batch c//4 and heads
[4*(c%4), 4*(c%4)+4) as 2 head-pairs. Host supplies xT [1024, 2048] bf16 per
batch and a [1024, 768] bf16 w-slice in 6 e-blocks [K_p0|Q_p0|V_p0|K_p1|
Q_p1|V_p1], each packing its pair's heads at columns 0-63 / 64-127.

v17 changes vs v16 (205.8us baseline):
  - Startup: w0/x0/x1 DMA'd in per-dc chunks interleaved so the first
    projection MM starts after ~200KB instead of ~2.8MB (PE start ~17us ->
    ~4us).
  - Epilogue off the PE: instead of PE-transposing OT [65,512] to get the
    softmax denominator onto partitions, compute rec = exp(-ln(den_row)) on
    ACT (both funcs in the natural_log_exp_and_others table set — Bacc
    subclass steers Exp there to avoid set thrash), broadcast it across
    partitions on GpSimd, and multiply on DVE. Output stays [d, q] in HBM;
    host transposes during unshard.
  - DMA gates moved from GpSimd to DVE so GpSimd only runs attn-lib
    custom instructions (single library load).
"""

import numpy as np
import ml_dtypes

import concourse.bass as bass
import concourse.tile as tile
from concourse import bacc, mybir
from concourse.bass_utils import run_bass_kernel_spmd
from concourse.hw_specs import get_activation_tables
from concourse.masks import make_identity
import bass_rust as _bass_rust

F32 = mybir.dt.float32
F32R = mybir.dt.float32r
BF16 = mybir.dt.bfloat16
EXP = mybir.ActivationFunctionType.Exp
LN = mybir.ActivationFunctionType.Ln

B = 2
N = 2048
DIM = 1024
HEADS = 16
HD = 64
NCORES = 8
H_LOC = 4
NPAIR = 2
QB = 512
NKB = 16


class _Bacc(bacc.Bacc):
    """Steer Exp onto the natural_log_exp_and_others ACT table set so the
    exp stream and the ln-based reciprocal share one set (no reloads)."""

    def insert_act_table_loads(self):
        has_activation = any(
            isinstance(i, mybir.InstActivation)
            for b in self.main_func.blocks
            for i in b.instructions
        )
        if not has_activation:
            return
        tables = []
        for name, funcs in get_activation_tables(self.m.arch).items():
            if name != "natural_log_exp_and_others":
                funcs = funcs - {EXP, LN}
            tables.append((name, funcs))
        _bass_rust.insert_act_table_loads(self, tables)


def _emit(tc, out_ap, x_ap, w_ap):
    nc = tc.nc
    from contextlib import ExitStack
    ctx = ExitStack()
    with ctx:
        const = ctx.enter_context(tc.tile_pool(name="const", bufs=1))
        xtp = ctx.enter_context(tc.tile_pool(name="xtp", bufs=1))
        wp = ctx.enter_context(tc.tile_pool(name="wp", bufs=1))
        qkvp = ctx.enter_context(tc.tile_pool(name="qkvp", bufs=1))
        vpp = ctx.enter_context(tc.tile_pool(name="vpp", bufs=1))
        ptp = ctx.enter_context(tc.tile_pool(name="ptp", bufs=1))
        osb = ctx.enter_context(tc.tile_pool(name="osb", bufs=4))
        onp = ctx.enter_context(tc.tile_pool(name="onp", bufs=3))
        smp = ctx.enter_context(tc.tile_pool(name="smp", bufs=4))

        ident = const.tile([128, 128], F32)
        make_identity(nc, ident)
        ident_r = const.tile([128, 128], F32R)
        nc.vector.tensor_copy(out=ident_r, in_=ident)
        ones_b = const.tile([128, 1], BF16)
        nc.vector.memset(ones_b, 1.0)

        # ---- input tiles. w0 / x0 / x1 are chunked per-dc so the first
        # projection waits only on its first ~200KB; the rest stream in
        # behind it. Later tensors are whole-tile and (for wave 2) gated.
        w0c = [wp.tile([128, 128], BF16, name=f"w0c{dc}") for dc in range(8)]
        w_e = [None] + [wp.tile([128, 8, 128], BF16, name=f"we{e}")
                        for e in range(1, 6)]
        x0c = [xtp.tile([128, 512], BF16, name=f"x0c{dc}") for dc in range(8)]
        x1c = [xtp.tile([128, 512], BF16, name=f"x1c{dc}") for dc in range(8)]
        xq = [None, None] + [xtp.tile([128, 8, 512], BF16, name=f"xq{qt}")
                             for qt in (2, 3)]

        def wv(e, dc):
            if e == 0:
                return w0c[dc]
            return w_e[e][:, dc, :]

        def xv(qt, dc):
            if qt == 0:
                return x0c[dc]
            if qt == 1:
                return x1c[dc]
            return xq[qt][:, dc, :]

        def dma_w(e):
            nc.sync.dma_start(
                out=w_e[e], in_=w_ap[e].rearrange("p (c f) -> p c f", f=128))

        def dma_x(qt):
            nc.sync.dma_start(
                out=xq[qt], in_=x_ap[qt].rearrange("p (c f) -> p c f", f=512))

        # first wave, interleaved fine chunks in need-order
        for dc in range(8):
            nc.sync.dma_start(out=w0c[dc],
                              in_=w_ap[0][:, dc * 128:(dc + 1) * 128])
            nc.sync.dma_start(out=x0c[dc],
                              in_=x_ap[0][:, dc * 512:(dc + 1) * 512])
        dma_w(1)
        for dc in range(8):
            nc.sync.dma_start(out=x1c[dc],
                              in_=x_ap[1][:, dc * 512:(dc + 1) * 512])
        dma_w(2)

        QKV = [qkvp.tile([128, N], F32R if e in (2, 5) else BF16,
                         name=f"qkv{e}") for e in range(6)]
        KT = [QKV[0], QKV[3]]
        QT = [QKV[1], QKV[4]]
        VT = [QKV[2], QKV[5]]
        Vp = [vpp.tile([128, NKB, 130], BF16, name=f"vp{p}")
              for p in range(NPAIR)]
        for p in range(NPAIR):
            nc.vector.memset(Vp[p], 1.0)   # ones columns 64/129 survive
        # PT-store for one mega-phase: 32 x [128, 1024] bf16 (64 KB/part)
        PTs = [ptp.tile([128, 1024], BF16, tag=f"PT{s}", name=f"pts{s}")
               for s in range(32)]

        # ---------------- PSUM layout (8 banks exactly) ----------------
        psS = ctx.enter_context(tc.tile_pool(name="psS", bufs=2, space="PSUM"))
        psO = ctx.enter_context(tc.tile_pool(name="psO", bufs=1, space="PSUM"))
        psP = ctx.enter_context(tc.tile_pool(name="psP", bufs=1, space="PSUM"))
        psT = ctx.enter_context(tc.tile_pool(name="psT", bufs=1, space="PSUM"))

        def mk_proj(e, qt):
            def run():
                ps = psP.tile([128, 512], F32, tag="pj", name=f"pj{e}{qt}")
                for dc in range(8):
                    nc.tensor.matmul(
                        ps, wv(e, dc), xv(qt, dc),
                        start=(dc == 0), stop=(dc == 7))
                nc.vector.tensor_copy(
                    out=QKV[e][:, qt * 512:(qt + 1) * 512], in_=ps)
            return run

        def mk_vtr(p, kb4):
            # transpose 4 V chunks (kb4*4 .. kb4*4+3) in one batch
            def run():
                tp = psT.tile([128, 4, 128], F32R, tag="tr", name=f"vt{p}{kb4}")
                for j in range(4):
                    kb = 4 * kb4 + j
                    nc.tensor.transpose(
                        tp[:, j, :], VT[p][:, kb * 128:(kb + 1) * 128],
                        ident_r)
                ks = slice(4 * kb4, 4 * kb4 + 4)
                nc.vector.tensor_copy(out=Vp[p][:, ks, 0:64],
                                      in_=tp[:, :, 0:64])
                nc.vector.tensor_copy(out=Vp[p][:, ks, 65:129],
                                      in_=tp[:, :, 64:128])
            return run

        def mk_epilog(p, qb, h, OT_sb):
            # OT_sb [65, 512]: rows 0-63 = O^T [d, q], row 64 = softmax
            # denominator (ones column is LAST in V'). rec = exp(-ln(den))
            # on ACT, broadcast down partitions on GpSimd, multiply on DVE.
            # HBM keeps [d, q]; host transposes during unshard.
            def run():
                lnd = smp.tile([1, 512], F32, tag="lnd")
                rec = smp.tile([1, 512], F32, tag="rec")
                rec64 = onp.tile([64, 512], F32, tag="r64",
                                 name=f"r64{p}{qb}{h}")
                o = onp.tile([64, 512], F32, tag="On", name=f"on{p}{qb}{h}")
                nc.scalar.activation(out=lnd, in_=OT_sb[64:65, :], func=LN)
                nc.scalar.activation(out=rec, in_=lnd, func=EXP, scale=-1.0)
                nc.gpsimd.partition_broadcast(rec64, rec)
                nc.vector.tensor_mul(out=o, in0=OT_sb[0:64, :], in1=rec64)
                nc.sync.dma_start(out=out_ap[2 * p + h, qb], in_=o)
            return run

        # ---- single strictly-ordered side queue (writer before reader) ----
        from collections import deque
        side = deque()
        side.append(("k0", mk_proj(0, 1)))    # K_p0 rest (qb0 kb4+)
        side.append(("k0", mk_proj(0, 2)))
        side.append(("k0", mk_proj(0, 3)))
        side.append(("q0", mk_proj(1, 1)))    # Q_p0 qt1 (qb1 S-phase)
        for qt in range(4):                   # V_p0 + its V' transposes
            side.append(("vtr0", mk_proj(2, qt)))
            side.append(("vtr0", mk_vtr(0, qt)))
        side.append(("q0", mk_proj(1, 2)))    # Q_p0 qt2, qt3 (mega 1)
        side.append(("q0", mk_proj(1, 3)))
        for e in (3, 4):                      # K_p1, Q_p1
            for qt in range(4):
                side.append(("kq1", mk_proj(e, qt)))
        for qt in range(4):                   # V_p1 + its V' transposes
            side.append(("vtr1", mk_proj(5, qt)))
            side.append(("vtr1", mk_vtr(1, qt)))

        def drain(label):
            while side and any(lab == label for lab, _ in side):
                side.popleft()[1]()

        # stage 0 (direct): first K and Q quarters
        mk_proj(0, 0)()
        mk_proj(1, 0)()
        # second DMA wave, gated behind stage 0 via a DVE read chain
        # (a prior reader of the DMA's out-tile forces the DMA to wait)
        gate = const.tile([1, 64], BF16, name="gate")
        nc.vector.tensor_copy(out=gate, in_=QKV[0][0:1, 0:64])
        for e in (3, 4, 5):
            nc.vector.tensor_copy(out=gate, in_=w_e[e][0:1, 0, 0:64])
            dma_w(e)
        for qt in (2, 3):
            nc.vector.tensor_copy(out=gate, in_=xq[qt][0:1, 0, 0:64])
            dma_x(qt)

        def emit_S_block(p, m, s_lo, s_hi):
            kT, qT = KT[p], QT[p]
            for s in range(s_lo, s_hi):
                qb = 2 * m + s // 16
                kb = s % 16
                qs = slice(qb * QB, (qb + 1) * QB)
                ks = slice(kb * 128, (kb + 1) * 128)
                S = psS.tile([128, 1024], F32, tag="S", name=f"s{p}{m}{s}")
                nc.tensor.matmul(S[:, 0:512], kT[0:64, ks], qT[0:64, qs],
                                 start=True, stop=True)
                nc.tensor.matmul(S[:, 512:1024], kT[64:128, ks],
                                 qT[64:128, qs], start=True, stop=True)
                if s % 4 == 1 and side:
                    side.popleft()[1]()
                    if side:
                        side.popleft()[1]()
                nc.scalar.activation(
                    out=PTs[s], in_=S, func=EXP, scale=0.125)

        def emit_PV_block(p, m, ql):
            qb = 2 * m + ql
            OTa = psO.tile([65, QB], F32, tag="OTa", name=f"ota{p}{qb}")
            OTb = psO.tile([65, QB], F32, tag="OTb", name=f"otb{p}{qb}")
            for kb in range(NKB):
                s = ql * 16 + kb
                nc.tensor.matmul(OTa, Vp[p][:, kb, 0:65],
                                 PTs[s][:, 0:512],
                                 start=(kb == 0), stop=(kb == NKB - 1))
                nc.tensor.matmul(OTb, Vp[p][:, kb, 65:130],
                                 PTs[s][:, 512:1024],
                                 start=(kb == 0), stop=(kb == NKB - 1))
                if kb % 4 == 2 and side and side[0][0] == "ep":
                    side.popleft()[1]()
            OT_sba = osb.tile([65, QB], F32, tag="OT", name=f"oa{p}{qb}")
            OT_sbb = osb.tile([65, QB], F32, tag="OT", name=f"ob{p}{qb}")
            nc.vector.tensor_copy(out=OT_sba, in_=OTa)
            nc.vector.tensor_copy(out=OT_sbb, in_=OTb)
            side.append(("ep", mk_epilog(p, qb, 0, OT_sba)))
            side.append(("ep", mk_epilog(p, qb, 1, OT_sbb)))

        megas = [(p, m) for p in range(NPAIR) for m in range(2)]
        PF = 14   # prefetch depth: exps write PTs[0..PF-1], all already
                  # consumed by this mega's PV ql0; PV ql1 reads PTs[16..31]
        for i, (p, m) in enumerate(megas):
            emit_S_block(p, m, PF if i > 0 else 0, 32)
            drain(f"vtr{p}")
            emit_PV_block(p, m, 0)
            if i + 1 < len(megas):
                pn, mn = megas[i + 1]
                if pn != p:
                    drain("kq1")
                # prefetch: next mega's first PF S-slots + exps keep the
                # scalar stream fed across the PV/mega + drain boundary
                emit_S_block(pn, mn, 0, PF)
            emit_PV_block(p, m, 1)
        while side:
            side.popleft()[1]()


_CACHED_NC = None


def _build():
    global _CACHED_NC
    if _CACHED_NC is not None:
        return _CACHED_NC
    nc = _Bacc("TRN2", target_bir_lowering=False, debug=False,
               num_devices=NCORES)
    x = nc.dram_tensor("x", [4, 128, 4096], BF16,
                       kind="ExternalInput").ap()
    w = nc.dram_tensor("w", [6, 128, 1024], BF16,
                       kind="ExternalInput").ap()
    out = nc.dram_tensor("out", [H_LOC, 4, HD, QB], F32,
                         kind="ExternalOutput").ap()
    with tile.TileContext(nc) as tc:
        _emit(tc, out, x, w)
    nc.compile()
    _CACHED_NC = nc
    return nc


def _w_slice(w_qkv: np.ndarray, core: int) -> np.ndarray:
    h0 = 4 * (core % 4)
    blocks = []
    for p in range(2):
        ha, hb = h0 + 2 * p, h0 + 2 * p + 1
        for part in (1, 0, 2):          # K, Q, V column groups of w_qkv
            base = part * HEADS * HD
            blocks.append(w_qkv[:, base + ha * HD: base + (ha + 1) * HD])
            blocks.append(w_qkv[:, base + hb * HD: base + (hb + 1) * HD])
    wc = np.concatenate(blocks, axis=1)            # [1024, 768]
    # SBUF tile layout: [e, p, c*128+f] = wc[c*128+p, e*128+f]
    w2 = wc.reshape(8, 128, 6, 128).transpose(2, 1, 0, 3).reshape(6, 128, 1024)
    return np.ascontiguousarray(w2).astype(ml_dtypes.bfloat16)


def kernel(x: np.ndarray, w_qkv: np.ndarray, _trace: bool = False):
    nc = _build()
    x = np.asarray(x, dtype=np.float32)
    w_qkv = np.asarray(w_qkv, dtype=np.float32)
    def xprep(b):
        xT = np.ascontiguousarray(x[b].T)          # [1024, 2048]
        x2 = xT.reshape(8, 128, 4, 512).transpose(2, 1, 0, 3)
        return np.ascontiguousarray(
            x2.reshape(4, 128, 4096)).astype(ml_dtypes.bfloat16)
    xs = [xprep(b) for b in range(B)]
    in_maps = [{"x": xs[c // 4], "w": _w_slice(w_qkv, c)}
               for c in range(NCORES)]
    res = run_bass_kernel_spmd(nc, in_maps, list(range(NCORES)), trace=_trace)
    out = np.empty((B, HEADS, N, HD), np.float32)
    for c in range(NCORES):
        b, h0 = c // 4, 4 * (c % 4)
        o = res.results[c]["out"]          # [4 heads, 4 qb, d, q]
        out[b, h0:h0 + 4] = o.transpose(0, 1, 3, 2).reshape(4, N, HD)
    if _trace:
        kernel.last_exec_time_ns = res.exec_time_ns
    return out


# revision 10
# speedup vs baseline: 1.0077x; 1.0077x over previous
"""Fused multi-head attention (QKV projection + softmax attention) on 8 TRN2
NeuronCores — v17.

Problem: x [2, 2048, 1024] f32, w_qkv [1024, 3072] f32 ->
         out [2, 16, 2048, 64] f32   (16 heads, head_dim 64)

Sharding: batch x head-group. Core c handles batch c//4 and heads
[4*(c%4), 4*(c%4)+4) as 2 head-pairs. Host supplies xT [1024, 2048] bf16 per
batch and a [1024, 768] bf16 w-slice in 6 e-blocks [K_p0|Q_p0|V_p0|K_p1|
Q_p1|V_p1], each packing its pair's heads at columns 0-63 / 64-127.

v17 changes vs v16 (205.8us baseline):
  - Startup: w0/x0/x1 DMA'd in per-dc chunks interleaved so the first
    projection MM starts after ~200KB instead of ~2.8MB (PE start ~17us ->
    ~4us).
  - Epilogue off the PE: instead of PE-transposing OT [65,512] to get the
    softmax denominator onto partitions, compute rec = exp(-ln(den_row)) on
    ACT (both funcs in the natural_log_exp_and_others table set — Bacc
    subclass steers Exp there to avoid set thrash), broadcast it across
    partitions on GpSimd, and multiply on DVE. Output stays [d, q] in HBM;
    host transposes during unshard.
  - DMA gates moved from GpSimd to DVE so GpSimd only runs attn-lib
    custom instructions (single library load).
"""

import numpy as np
import ml_dtypes

import concourse.bass as bass
import concourse.tile as tile
from concourse import bacc, mybir
from concourse.bass_utils import run_bass_kernel_spmd
from concourse.hw_specs import get_activation_tables
from concourse.masks import make_identity
import bass_rust as _bass_rust

F32 = mybir.dt.float32
F32R = mybir.dt.float32r
BF16 = mybir.dt.bfloat16
EXP = mybir.ActivationFunctionType.Exp
LN = mybir.ActivationFunctionType.Ln

B = 2
N = 2048
DIM = 1024
HEADS = 16
HD = 64
NCORES = 8
H_LOC = 4
NPAIR = 2
QB = 512
NKB = 16


class _Bacc(bacc.Bacc):
    """Steer Exp onto the natural_log_exp_and_others ACT table set so the
    exp stream and the ln-based reciprocal share one set (no reloads)."""

    def insert_act_table_loads(self):
        has_activation = any(
            isinstance(i, mybir.InstActivation)
            for b in self.main_func.blocks
            for i in b.instructions
        )
        if not has_activation:
            return
        tables = []
        for name, funcs in get_activation_tables(self.m.arch).items():
            if name != "natural_log_exp_and_others":
                funcs = funcs - {EXP, LN}
            tables.append((name, funcs))
        _bass_rust.insert_act_table_loads(self, tables)


def _emit(tc, out_ap, x_ap, w_ap):
    nc = tc.nc
    from contextlib import ExitStack
    ctx = ExitStack()
    with ctx:
        const = ctx.enter_context(tc.tile_pool(name="const", bufs=1))
        xtp = ctx.enter_context(tc.tile_pool(name="xtp", bufs=1))
        wp = ctx.enter_context(tc.tile_pool(name="wp", bufs=1))
        qkvp = ctx.enter_context(tc.tile_pool(name="qkvp", bufs=1))
        vpp = ctx.enter_context(tc.tile_pool(name="vpp", bufs=1))
        ptp = ctx.enter_context(tc.tile_pool(name="ptp", bufs=1))
        osb = ctx.enter_context(tc.tile_pool(name="osb", bufs=4))
        onp = ctx.enter_context(tc.tile_pool(name="onp", bufs=3))
        smp = ctx.enter_context(tc.tile_pool(name="smp", bufs=2))

        ident = const.tile([128, 128], F32)
        make_identity(nc, ident)
        ident_r = const.tile([128, 128], F32R)
        nc.vector.tensor_copy(out=ident_r, in_=ident)
        ones_b = const.tile([128, 1], BF16)
        nc.vector.memset(ones_b, 1.0)

        # ---- input tiles. w0 / x0 / x1 are chunked per-dc so the first
        # projection waits only on its first ~200KB; the rest stream in
        # behind it. Later tensors are whole-tile and (for wave 2) gated.
        w0c = [wp.tile([128, 128], BF16, name=f"w0c{dc}") for dc in range(8)]
        w_e = [None] + [wp.tile([128, 8, 128], BF16, name=f"we{e}")
                        for e in range(1, 6)]
        x0c = [xtp.tile([128, 512], BF16, name=f"x0c{dc}") for dc in range(8)]
        x1c = [xtp.tile([128, 512], BF16, name=f"x1c{dc}") for dc in range(8)]
        xq = [None, None] + [xtp.tile([128, 8, 512], BF16, name=f"xq{qt}")
                             for qt in (2, 3)]

        def wv(e, dc):
            if e == 0:
                return w0c[dc]
            return w_e[e][:, dc, :]

        def xv(qt, dc):
            if qt == 0:
                return x0c[dc]
            if qt == 1:
                return x1c[dc]
            return xq[qt][:, dc, :]

        def dma_w(e):
            nc.sync.dma_start(
                out=w_e[e], in_=w_ap[e].rearrange("p (c f) -> p c f", f=128))

        def dma_x(qt):
            nc.sync.dma_start(
                out=xq[qt], in_=x_ap[qt].rearrange("p (c f) -> p c f", f=512))

        # first wave, interleaved fine chunks in need-order
        for dc in range(8):
            nc.sync.dma_start(out=w0c[dc],
                              in_=w_ap[0][:, dc * 128:(dc + 1) * 128])
            nc.sync.dma_start(out=x0c[dc],
                              in_=x_ap[0][:, dc * 512:(dc + 1) * 512])
        dma_w(1)
        for dc in range(8):
            nc.sync.dma_start(out=x1c[dc],
                              in_=x_ap[1][:, dc * 512:(dc + 1) * 512])
        dma_w(2)

        QKV = [qkvp.tile([128, N], F32R if e in (2, 5) else BF16,
                         name=f"qkv{e}") for e in range(6)]
        KT = [QKV[0], QKV[3]]
        QT = [QKV[1], QKV[4]]
        VT = [QKV[2], QKV[5]]
        Vp = [vpp.tile([128, NKB, 130], BF16, name=f"vp{p}")
              for p in range(NPAIR)]
        for p in range(NPAIR):
            nc.vector.memset(Vp[p], 1.0)   # ones columns 64/129 survive
        # PT-store for one mega-phase: 32 x [128, 1024] bf16 (64 KB/part)
        PTs = [ptp.tile([128, 1024], BF16, tag=f"PT{s}", name=f"pts{s}")
               for s in range(32)]

        # ---------------- PSUM layout (8 banks exactly) ----------------
        psS = ctx.enter_context(tc.tile_pool(name="psS", bufs=2, space="PSUM"))
        psO = ctx.enter_context(tc.tile_pool(name="psO", bufs=1, space="PSUM"))
        psP = ctx.enter_context(tc.tile_pool(name="psP", bufs=1, space="PSUM"))
        psT = ctx.enter_context(tc.tile_pool(name="psT", bufs=1, space="PSUM"))

        def mk_proj(e, qt):
            def run():
                ps = psP.tile([128, 512], F32, tag="pj", name=f"pj{e}{qt}")
                for dc in range(8):
                    nc.tensor.matmul(
                        ps, wv(e, dc), xv(qt, dc),
                        start=(dc == 0), stop=(dc == 7))
                nc.vector.tensor_copy(
                    out=QKV[e][:, qt * 512:(qt + 1) * 512], in_=ps)
            return run

        def mk_vtr(p, kb4):
            # transpose 4 V chunks (kb4*4 .. kb4*4+3) in one batch
            def run():
                tp = psT.tile([128, 4, 128], F32R, tag="tr", name=f"vt{p}{kb4}")
                for j in range(4):
                    kb = 4 * kb4 + j
                    nc.tensor.transpose(
                        tp[:, j, :], VT[p][:, kb * 128:(kb + 1) * 128],
                        ident_r)
                ks = slice(4 * kb4, 4 * kb4 + 4)
                nc.vector.tensor_copy(out=Vp[p][:, ks, 0:64],
                                      in_=tp[:, :, 0:64])
                nc.vector.tensor_copy(out=Vp[p][:, ks, 65:129],
                                      in_=tp[:, :, 64:128])
            return run

        def mk_recip2(dens2, rec2, recB):
            # batched softmax-denominator reciprocal for one PV block's two
            # heads: rec2[h] = exp(-ln(dens2[h])). Emitted as a side item so
            # the gather DMAs have landed before these enter the ACT FIFO
            # (strict FIFO — a stalled ln would head-of-line block the exps).
            # Engine APs need 32-aligned base partitions, so row 1 is DMA'd
            # to the base-0 tile recB for the h=1 partition_broadcast.
            def run():
                lnd2 = smp.tile([2, 512], F32, tag="lnd")
                nc.scalar.activation(out=lnd2, in_=dens2, func=LN)
                nc.scalar.activation(out=rec2, in_=lnd2, func=EXP, scale=-1.0)
                nc.sync.dma_start(out=recB, in_=rec2[1:2, :])
            return run

        def mk_epilog(p, qb, h, OT_sb, rec_row, direct):
            # OT_sb [65, 512]: rows 0-63 = O^T [d, q], row 64 = softmax
            # denominator (ones column is LAST in V'). Broadcast the
            # reciprocal row down partitions on GpSimd, multiply on DVE.
            # HBM keeps [d, q]; host transposes during unshard.
            def run():
                if direct:
                    # last block: per-OT ln/exp straight off OT_sb (no DMA
                    # hop on the tail chain; ACT is idle by then)
                    lnd = smp.tile([1, 512], F32, tag="lnd1")
                    rr = smp.tile([1, 512], F32, tag="rec1")
                    nc.scalar.activation(out=lnd, in_=OT_sb[64:65, :],
                                         func=LN)
                    nc.scalar.activation(out=rr, in_=lnd, func=EXP,
                                         scale=-1.0)
                else:
                    rr = rec_row
                rec64 = onp.tile([64, 512], F32, tag="r64",
                                 name=f"r64{p}{qb}{h}")
                o = onp.tile([64, 512], F32, tag="On", name=f"on{p}{qb}{h}")
                nc.gpsimd.partition_broadcast(rec64, rr)
                nc.vector.tensor_mul(out=o, in0=OT_sb[0:64, :], in1=rec64)
                nc.sync.dma_start(out=out_ap[2 * p + h, qb], in_=o)
            return run

        # ---- single strictly-ordered side queue (writer before reader) ----
        from collections import deque
        side = deque()
        side.append(("k0", mk_proj(0, 1)))    # K_p0 rest (qb0 kb4+)
        side.append(("k0", mk_proj(0, 2)))
        side.append(("k0", mk_proj(0, 3)))
        side.append(("q0", mk_proj(1, 1)))    # Q_p0 qt1 (qb1 S-phase)
        for qt in range(4):                   # V_p0 + its V' transposes
            side.append(("vtr0", mk_proj(2, qt)))
            side.append(("vtr0", mk_vtr(0, qt)))
        side.append(("q0", mk_proj(1, 2)))    # Q_p0 qt2, qt3 (mega 1)
        side.append(("q0", mk_proj(1, 3)))
        for e in (3, 4):                      # K_p1, Q_p1
            for qt in range(4):
                side.append(("kq1", mk_proj(e, qt)))
        for qt in range(4):                   # V_p1 + its V' transposes
            side.append(("vtr1", mk_proj(5, qt)))
            side.append(("vtr1", mk_vtr(1, qt)))

        def drain(label):
            while side and any(lab == label for lab, _ in side):
                side.popleft()[1]()

        # stage 0 (direct): first K and Q quarters
        mk_proj(0, 0)()
        mk_proj(1, 0)()
        # second DMA wave, gated behind stage 0 via a DVE read chain
        # (a prior reader of the DMA's out-tile forces the DMA to wait)
        gate = const.tile([1, 64], BF16, name="gate")
        nc.vector.tensor_copy(out=gate, in_=QKV[0][0:1, 0:64])
        for e in (3, 4, 5):
            nc.vector.tensor_copy(out=gate, in_=w_e[e][0:1, 0, 0:64])
            dma_w(e)
        for qt in (2, 3):
            nc.vector.tensor_copy(out=gate, in_=xq[qt][0:1, 0, 0:64])
            dma_x(qt)

        def emit_S_slot(p, m, s):
            kT, qT = KT[p], QT[p]
            qb = 2 * m + s // 16
            kb = s % 16
            qs = slice(qb * QB, (qb + 1) * QB)
            ks = slice(kb * 128, (kb + 1) * 128)
            S = psS.tile([128, 1024], F32, tag="S", name=f"s{p}{m}{s}")
            nc.tensor.matmul(S[:, 0:512], kT[0:64, ks], qT[0:64, qs],
                             start=True, stop=True)
            nc.tensor.matmul(S[:, 512:1024], kT[64:128, ks],
                             qT[64:128, qs], start=True, stop=True)
            if s % 4 == 1 and side:
                side.popleft()[1]()
                if side:
                    side.popleft()[1]()
            nc.scalar.activation(
                out=PTs[s], in_=S, func=EXP, scale=0.125)

        def emit_PV_block(p, m, ql, carry, nxt, last=False):
            # carry: one S slot of the *next* mega left over from the
            # previous block — emitted first so the PE has work while the
            # previous block's OT psum drains through the DVE copy (psO has
            # a single buffer). nxt: this block's 16 next-mega S slots;
            # slot k is emitted after PV kb=k has consumed PTs[k] (WAR).
            # Returns the new carry (last slot of nxt).
            qb = 2 * m + ql
            if carry is not None:
                emit_S_slot(*carry)
            OTa = psO.tile([65, QB], F32, tag="OTa", name=f"ota{p}{qb}")
            OTb = psO.tile([65, QB], F32, tag="OTb", name=f"otb{p}{qb}")
            for kb in range(NKB):
                s = ql * 16 + kb
                nc.tensor.matmul(OTa, Vp[p][:, kb, 0:65],
                                 PTs[s][:, 0:512],
                                 start=(kb == 0), stop=(kb == NKB - 1))
                nc.tensor.matmul(OTb, Vp[p][:, kb, 65:130],
                                 PTs[s][:, 512:1024],
                                 start=(kb == 0), stop=(kb == NKB - 1))
                if kb % 4 == 2 and side:
                    side.popleft()[1]()
                if nxt and kb >= 1:
                    emit_S_slot(*nxt[kb - 1])
            OT_sba = osb.tile([65, QB], F32, tag="OT", name=f"oa{p}{qb}")
            OT_sbb = osb.tile([65, QB], F32, tag="OT", name=f"ob{p}{qb}")
            nc.vector.tensor_copy(out=OT_sba, in_=OTa)
            nc.vector.tensor_copy(out=OT_sbb, in_=OTb)
            if last:
                side.append(("ep", mk_epilog(p, qb, 0, OT_sba, None, True)))
                side.append(("ep", mk_epilog(p, qb, 1, OT_sbb, None, True)))
            else:
                # gather the two denominator rows onto partitions 0/1 of a
                # collector (cross-partition: DMA), then one batched ln/exp
                # pair on ACT via the side queue (DMAs land first)
                dens2 = smp.tile([2, 512], F32, tag="d2", name=f"d2{p}{qb}")
                rec2 = smp.tile([2, 512], F32, tag="r2", name=f"r2{p}{qb}")
                recB = smp.tile([1, 512], F32, tag="rB", name=f"rB{p}{qb}")
                nc.sync.dma_start(out=dens2[0:1, :], in_=OT_sba[64:65, :])
                nc.sync.dma_start(out=dens2[1:2, :], in_=OT_sbb[64:65, :])
                side.append(("ep", mk_recip2(dens2, rec2, recB)))
                side.append(("ep", mk_epilog(p, qb, 0, OT_sba,
                                             rec2[0:1, :], False)))
                side.append(("ep", mk_epilog(p, qb, 1, OT_sbb,
                                             recB, False)))
            return nxt[NKB - 1] if nxt else None

        megas = [(p, m) for p in range(NPAIR) for m in range(2)]
        carry = None
        for i, (p, m) in enumerate(megas):
            if i == 0:
                for s in range(32):
                    emit_S_slot(p, m, s)
            drain(f"vtr{p}")
            if i + 1 < len(megas):
                pn, mn = megas[i + 1]
                if pn != p:
                    drain("kq1")
                nxt0 = [(pn, mn, s) for s in range(16)]
                nxt1 = [(pn, mn, s) for s in range(16, 32)]
            else:
                nxt0 = nxt1 = None
            last = i + 1 == len(megas)
            carry = emit_PV_block(p, m, 0, carry, nxt0, last=False)
            carry = emit_PV_block(p, m, 1, carry, nxt1, last=last)
        while side:
            side.popleft()[1]()


_CACHED_NC = None


def _build():
    global _CACHED_NC
    if _CACHED_NC is not None:
        return _CACHED_NC
    nc = _Bacc("TRN2", target_bir_lowering=False, debug=False,
               num_devices=NCORES)
    x = nc.dram_tensor("x", [4, 128, 4096], BF16,
                       kind="ExternalInput").ap()
    w = nc.dram_tensor("w", [6, 128, 1024], BF16,
                       kind="ExternalInput").ap()
    out = nc.dram_tensor("out", [H_LOC, 4, HD, QB], F32,
                         kind="ExternalOutput").ap()
    with tile.TileContext(nc) as tc:
        _emit(tc, out, x, w)
    nc.compile()
    _CACHED_NC = nc
    return nc


def _w_slice(w_qkv: np.ndarray, core: int) -> np.ndarray:
    h0 = 4 * (core % 4)
    blocks = []
    for p in range(2):
        ha, hb = h0 + 2 * p, h0 + 2 * p + 1
        for part in (1, 0, 2):          # K, Q, V column groups of w_qkv
            base = part * HEADS * HD
            blocks.append(w_qkv[:, base + ha * HD: base + (ha + 1) * HD])
            blocks.append(w_qkv[:, base + hb * HD: base + (hb + 1) * HD])
    wc = np.concatenate(blocks, axis=1)            # [1024, 768]
    # SBUF tile layout: [e, p, c*128+f] = wc[c*128+p, e*128+f]
    w2 = wc.reshape(8, 128, 6, 128).transpose(2, 1, 0, 3).reshape(6, 128, 1024)
    return np.ascontiguousarray(w2).astype(ml_dtypes.bfloat16)


def kernel(x: np.ndarray, w_qkv: np.ndarray, _trace: bool = False):
    nc = _build()
    x = np.asarray(x, dtype=np.float32)
    w_qkv = np.asarray(w_qkv, dtype=np.float32)
    def xprep(b):
        xT = np.ascontiguousarray(x[b].T)          # [1024, 2048]
        x2 = xT.reshape(8, 128, 4, 512).transpose(2, 1, 0, 3)
        return np.ascontiguousarray(
            x2.reshape(4, 128, 4096)).astype(ml_dtypes.bfloat16)
    xs = [xprep(b) for b in range(B)]
    in_maps = [{"x": xs[c // 4], "w": _w_slice(w_qkv, c)}
               for c in range(NCORES)]
    res = run_bass_kernel_spmd(nc, in_maps, list(range(NCORES)), trace=_trace)
    out = np.empty((B, HEADS, N, HD), np.float32)
    for c in range(NCORES):
        b, h0 = c // 4, 4 * (c % 4)
        o = res.results[c]["out"]          # [4 heads, 4 qb, d, q]
        out[b, h0:h0 + 4] = o.transpose(0, 1, 3, 2).reshape(4, N, HD)
    if _trace:
        kernel.last_exec_time_ns = res.exec_time_ns
    return out
